# revision 1
# baseline (speedup 1.0000x reference)
"""CKAM (DANet-style dual attention) Bass kernel for 8 trn2 NeuronCores.

Data-parallel over batch: each core processes one [512, 64, 64] image.

Per-core dataflow (N = H*W = 4096, C = 512, CH = 256, R = 64):
  Phase A: packed conv  [q|k](128, N)  = Wsp^T  @ [top;bottom]   (spatial q, k)
           chunk-outer accumulation, overlaps the input DMA stream.
  Phase B: transposed conv (N, 192) = [top;bottom]^T @ Wcsc      (qc^T, kc^T, ks^T)
  Phase C: conv          kc(64, N)  = Wkc^T @ [top;bottom]       (channel k)
  Channel attn:  scores = qc @ kc^T  (64x64), softmax, out_c = attn @ kc
  Spatial attn:  chunk pairs (even on PE rows 0:64, odd on rows 64:128 for
                 row-group concurrency): S = q^T k -> exp (ACT, accum d) ->
                 out_sp += (ks^T / d) contracted with E (col-group pairs)
  Final: out = [fs|fc] @ [out_sp; out_c] + bias   (single K=128 conv)

All 1x1 convs are folded through the (never materialized) x = top+bottom:
composite weights are computed on the host in float64.
"""

import numpy as np

import concourse.bass as bass
import concourse.bacc as bacc
import concourse.mybir as mybir
import concourse.tile as tile
from concourse import bass_utils
from concourse.bass import ts
from concourse.masks import make_identity

N_CORES = 8
C, HW = 512, 4096
CH, R = 256, 64
F32 = mybir.dt.float32
BF16 = mybir.dt.bfloat16
F32R = mybir.dt.float32r
EXP = mybir.ActivationFunctionType.Exp
AX = mybir.AxisListType.X

_CACHE: dict = {}

# Load top/bottom as bf16 (halves input DMA; rel err ~5e-3 vs ~2e-3)
INPUT_BF16 = True

ALL_PHASES = ("pa", "pb", "pc", "chan", "spatial", "final")


def build_program(phases=ALL_PHASES, repeat=1, input_bf16=None):
    if input_bf16 is None:
        input_bf16 = INPUT_BF16
    WDT = BF16 if input_bf16 else F32R
    WB = 192 if input_bf16 else 256
    IDT = BF16 if input_bf16 else F32R
    nc = bacc.Bacc("TRN2", target_bir_lowering=False, debug=False)

    top = nc.dram_tensor("top", (C, HW), IDT, kind="ExternalInput").ap()
    bot = nc.dram_tensor("bot", (C, HW), IDT, kind="ExternalInput").ap()
    wsp = nc.dram_tensor("wsp", (128, 8, 128), WDT, kind="ExternalInput").ap()
    wcsc = nc.dram_tensor("wcsc", (128, 8, WB), WDT, kind="ExternalInput").ap()
    wfa = nc.dram_tensor("wfa", (128, 4, 128), F32R, kind="ExternalInput").ap()
    wfb = nc.dram_tensor("wfb", (128, 4, 128), F32R, kind="ExternalInput").ap()
    b_qk = nc.dram_tensor("b_qk", (128, 1), F32, kind="ExternalInput").ap()
    b_csc = nc.dram_tensor("b_csc", (128, 192), F32, kind="ExternalInput").ap()
    b_kc = nc.dram_tensor("b_kc", (64, 1), F32, kind="ExternalInput").ap()
    b_f = nc.dram_tensor("b_f", (128, 4), F32, kind="ExternalInput").ap()
    out_d = nc.dram_tensor("out", (C, HW), F32, kind="ExternalOutput").ap()

    with tile.TileContext(nc) as tc:
      for _rep in range(repeat):
        with (
            tc.tile_pool(name="consts", bufs=1) as consts,
            tc.tile_pool(name="persist", bufs=1) as persist,
        ):
            wsp_sb = consts.tile([128, 8, 128], WDT)
            nc.sync.dma_start(out=wsp_sb, in_=wsp)
            wcsc_sb = consts.tile([128, 8, WB], WDT)
            nc.sync.dma_start(out=wcsc_sb, in_=wcsc)
            wfa_sb = consts.tile([128, 4, 128], F32R)
            nc.sync.dma_start(out=wfa_sb, in_=wfa)
            wfb_sb = consts.tile([128, 4, 128], F32R)
            nc.sync.dma_start(out=wfb_sb, in_=wfb)
            bqk_sb = consts.tile([128, 1], F32)
            nc.sync.dma_start(out=bqk_sb, in_=b_qk)
            bcsc_sb = consts.tile([128, 192], F32)
            nc.sync.dma_start(out=bcsc_sb, in_=b_csc)
            bkc_sb = consts.tile([64, 1], F32)
            nc.sync.dma_start(out=bkc_sb, in_=b_kc)
            bf_sb = consts.tile([128, 4], F32)
            nc.sync.dma_start(out=bf_sb, in_=b_f)
            ident = consts.tile([64, 64], F32)
            make_identity(nc, ident)

            # conv-phase outputs that live through the attention phase
            qk_sb = persist.tile([128, HW], BF16)  # q rows 0:64, k rows 64:128
            qk_swap = persist.tile([128, HW], BF16)  # [k | q] partition-swapped
            qckcT = persist.tile([128, 32, 128], F32)  # qc^T | kc^T  (n-major)
            ksT = persist.tile([128, 32, 64], BF16)  # spatial k^T
            kc_sb = persist.tile([64, HW], BF16)  # channel k
            stacked = persist.tile([128, HW], F32R)  # [out_sp|out_c] (swapped odd mb)

            # ---------------- conv phases (inputs resident) ----------------
            with tc.tile_pool(name="inputs", bufs=1) as inputs:
                top_r = top.rearrange("(a p) m -> a p m", p=128)
                bot_r = bot.rearrange("(a p) m -> a p m", p=128)
                srcs = [top_r[a] for a in range(4)] + [bot_r[a] for a in range(4)]
                if input_bf16:
                    chunks = []
                    for ci in range(8):
                        ch = inputs.tile([128, HW], BF16, tag=f"ch{ci}",
                                         name=f"ch{ci}")
                        nc.sync.dma_start(out=ch, in_=srcs[ci])
                        chunks.append(ch)
                else:
                    top_sb = inputs.tile([128, 4, HW], F32R)
                    bot_sb = inputs.tile([128, 4, HW], F32R)
                    for a in range(4):
                        nc.sync.dma_start(out=top_sb[:, a, :], in_=top_r[a])
                        nc.sync.dma_start(out=bot_sb[:, a, :], in_=bot_r[a])
                    chunks = [top_sb[:, a, :] for a in range(4)] + [
                        bot_sb[:, a, :] for a in range(4)
                    ]

                # Phase A (chunk-outer: starts as soon as chunk 0 lands)
                if "pa" in phases:
                    with tc.tile_pool(name="psA", bufs=1, space="PSUM") as psA:
                        psa_t = [
                            psA.tile([128, 512], F32, tag=f"a{mb}", name=f"psa{mb}")
                            for mb in range(8)
                        ]
                        for ci in range(8):
                            for mb in range(8):
                                nc.tensor.matmul(
                                    psa_t[mb],
                                    wsp_sb[:, ci, :],
                                    chunks[ci][:, ts(mb, 512)],
                                    start=(ci == 0),
                                    stop=(ci == 7),
                                )
                        for mb in range(8):
                            nc.vector.tensor_scalar_add(
                                qk_sb[:, ts(mb, 512)], psa_t[mb], bqk_sb
                            )
                    # [k|q] partition-swapped duplicate (SBUF->SBUF DMA)
                    nc.sync.dma_start(out=qk_swap[0:64, :], in_=qk_sb[64:128, :])
                    nc.sync.dma_start(out=qk_swap[64:128, :], in_=qk_sb[0:64, :])

                # Phase B: transposed convs -> qc^T | kc^T | ks^T
                if "pb" in phases:
                    with tc.tile_pool(name="psB", bufs=4, space="PSUM") as psB:
                        for nb in range(32):
                            ps = psB.tile([128, WB], F32, tag="b")
                            for ci in range(8):
                                nc.tensor.matmul(
                                    ps,
                                    chunks[ci][:, ts(nb, 128)],
                                    wcsc_sb[:, ci, :],
                                    start=(ci == 0),
                                    stop=(ci == 7),
                                )
                            nc.vector.tensor_add(
                                qckcT[:, nb, :], ps[:, 0:128], bcsc_sb[:, 0:128]
                            )
                            nc.vector.tensor_add(
                                ksT[:, nb, :], ps[:, 128:192], bcsc_sb[:, 128:192]
                            )

                # Phase C: channel kc conv -> [64, HW]
                if "pc" in phases:
                    with tc.tile_pool(name="psC", bufs=4, space="PSUM") as psC:
                        for mb in range(8):
                            ps = psC.tile([64, 512], F32, tag="c")
                            for ci in range(8):
                                nc.tensor.matmul(
                                    ps,
                                    wcsc_sb[:, ci, 64:128],
                                    chunks[ci][:, ts(mb, 512)],
                                    start=(ci == 0),
                                    stop=(ci == 7),
                                )
                            nc.vector.tensor_scalar_add(
                                kc_sb[:, ts(mb, 512)], ps, bkc_sb
                            )

            # ---------------- channel attention ----------------
            if "chan" not in phases:
                nc.vector.memset(stacked.bitcast(F32), 0.0)
            if "chan" in phases:
                with (
                    tc.tile_pool(name="chan", bufs=1) as chs,
                    tc.tile_pool(name="chp", bufs=1, space="PSUM") as chp,
                    tc.tile_pool(name="chop", bufs=2, space="PSUM") as chop,
                ):
                    sc_ps = chp.tile([64, 64], F32, tag="sc")
                    for nb in range(32):
                        nc.tensor.matmul(
                            sc_ps,
                            qckcT[:, nb, 0:64],
                            qckcT[:, nb, 64:128],
                            start=(nb == 0),
                            stop=(nb == 31),
                        )
                    sc = chs.tile([64, 64], F32)
                    nc.vector.tensor_copy(sc, sc_ps)
                    mx = chs.tile([64, 1], F32)
                    nc.vector.reduce_max(mx, sc, axis=AX)
                    negmx = chs.tile([64, 1], F32)
                    nc.vector.tensor_scalar_mul(negmx, mx, -1.0)
                    ec = chs.tile([64, 64], F32)
                    dc = chs.tile([64, 1], F32)
                    nc.scalar.activation(
                        ec, sc, EXP, bias=negmx, scale=1.0, accum_out=dc
                    )
                    rdc = chs.tile([64, 1], F32)
                    nc.vector.reciprocal(rdc, dc)
                    ac = chs.tile([64, 64], F32)
                    nc.vector.tensor_scalar_mul(ac, ec, rdc)
                    acT_ps = chp.tile([64, 64], F32, tag="acT")
                    nc.tensor.transpose(acT_ps, ac, ident)
                    acT = chs.tile([64, 64], BF16)
                    nc.vector.tensor_copy(acT, acT_ps)
                    # out_c: even mb -> partitions 64:128, odd mb -> 0:64
                    for j in range(4):
                        ps = chop.tile([128, 512], F32, tag="oc")
                        nc.tensor.matmul(
                            ps[64:128, :], acT, kc_sb[:, ts(2 * j, 512)],
                            start=True, stop=True, skip_group_check=True,
                        )
                        nc.tensor.matmul(
                            ps[0:64, :], acT, kc_sb[:, ts(2 * j + 1, 512)],
                            start=True, stop=True, skip_group_check=True,
                        )
                        nc.vector.tensor_copy(
                            stacked[64:128, ts(2 * j, 512)], ps[64:128, :]
                        )
                        nc.vector.tensor_copy(
                            stacked[0:64, ts(2 * j + 1, 512)], ps[0:64, :]
                        )

            # ---------------- spatial attention ----------------
            if "spatial" in phases:
                with (
                    tc.tile_pool(name="spE", bufs=2) as spp,
                    tc.tile_pool(name="spS", bufs=2) as sps,
                    tc.tile_pool(name="psSe", bufs=1, space="PSUM") as psSe,
                    tc.tile_pool(name="psSo", bufs=1, space="PSUM") as psSo,
                    tc.tile_pool(name="psO", bufs=1, space="PSUM") as psO,
                ):
                    out_ps = [
                        psO.tile([128, 512], F32, tag=f"o{j}", name=f"out_ps{j}")
                        for j in range(4)
                    ]
                    # chunk pairs: even chunk on PE rows 0:64, odd on 64:128
                    for t in range(16):
                        i_e, i_o = 2 * t, 2 * t + 1
                        E_e = spp.tile([128, HW], BF16, tag="Ee", name="E_e")
                        E_o = spp.tile([128, HW], BF16, tag="Eo", name="E_o")
                        dp_e = sps.tile([128, 4], F32, tag="dpe", name="dp_e")
                        dp_o = sps.tile([128, 4], F32, tag="dpo", name="dp_o")
                        for q in range(4):
                            s_e = psSe.tile([128, 1024], F32, tag="se", name="s_e")
                            s_o = psSo.tile([128, 1024], F32, tag="so", name="s_o")
                            for jm in range(2):
                                mb = 2 * q + jm
                                nc.tensor.matmul(
                                    s_e[:, ts(jm, 512)],
                                    qk_sb[0:64, ts(i_e, 128)],
                                    qk_swap[0:64, ts(mb, 512)],
                                    start=True,
                                    stop=True,
                                )
                                nc.tensor.matmul(
                                    s_o[:, ts(jm, 512)],
                                    qk_swap[64:128, ts(i_o, 128)],
                                    qk_sb[64:128, ts(mb, 512)],
                                    start=True,
                                    stop=True,
                                )
                            nc.scalar.activation(
                                E_e[:, ts(q, 1024)], s_e, EXP,
                                accum_out=dp_e[:, q : q + 1],
                            )
                            nc.scalar.activation(
                                E_o[:, ts(q, 1024)], s_o, EXP,
                                accum_out=dp_o[:, q : q + 1],
                            )
                        for par, i_c, dp, E in (
                            ("e", i_e, dp_e, E_e),
                            ("o", i_o, dp_o, E_o),
                        ):
                            d = sps.tile([128, 1], F32, tag=f"d{par}", name="d")
                            nc.vector.reduce_sum(d, dp, axis=AX)
                            rd = sps.tile([128, 1], F32, tag=f"rd{par}", name="rd")
                            nc.vector.reciprocal(rd, d)
                            kst = sps.tile(
                                [128, 64], BF16, tag=f"kst{par}", name="kst"
                            )
                            nc.vector.tensor_scalar_mul(kst, ksT[:, i_c, :], rd)
                            for j in range(4):
                                nc.tensor.matmul(
                                    out_ps[j][0:64, :], kst, E[:, ts(2 * j, 512)],
                                    start=(i_c == 0), stop=(i_c == 31),
                                    skip_group_check=True,
                                )
                                nc.tensor.matmul(
                                    out_ps[j][64:128, :],
                                    kst,
                                    E[:, ts(2 * j + 1, 512)],
                                    start=(i_c == 0), stop=(i_c == 31),
                                    skip_group_check=True,
                                )
                    for j in range(4):
                        nc.vector.tensor_copy(
                            stacked[0:64, ts(2 * j, 512)], out_ps[j][0:64, :]
                        )
                        nc.vector.tensor_copy(
                            stacked[64:128, ts(2 * j + 1, 512)],
                            out_ps[j][64:128, :],
                        )

            # ---------------- final fused conv ----------------
            if "final" in phases:
                with (
                    tc.tile_pool(name="fin", bufs=4) as fins,
                    tc.tile_pool(name="psF", bufs=4, space="PSUM") as psF,
                ):
                    out_r = out_d.rearrange("(k p) m -> k p m", p=128)
                    for mb in range(8):
                        wf = wfa_sb if mb % 2 == 0 else wfb_sb
                        for cok in range(4):
                            ps = psF.tile([128, 512], F32, tag="f")
                            nc.tensor.matmul(
                                ps,
                                wf[:, cok, :],
                                stacked[:, ts(mb, 512)],
                                start=True, stop=True,
                            )
                            ft = fins.tile([128, 512], F32, tag="ft")
                            if (mb * 4 + cok) % 2 == 0:
                                nc.vector.tensor_scalar_add(
                                    ft, ps, bf_sb[:, cok : cok + 1]
                                )
                            else:
                                nc.scalar.add(ft, ps, bf_sb[:, cok : cok + 1])
                            nc.sync.dma_start(
                                out=out_r[cok, :, ts(mb, 512)], in_=ft
                            )

    nc.compile()
    return nc


def make_weight_arrays(inputs):
    """Host-side composite weights (float64 accumulate, float32 out)."""
    f8 = lambda a: np.asarray(a, dtype=np.float64)
    wt, wb = f8(inputs["wt"]), f8(inputs["wb"])
    bt, bb = f8(inputs["bt"]), f8(inputs["bb"])
    s_w1, s_b1 = f8(inputs["s_w1"]), f8(inputs["s_b1"])
    s_w2, s_b2 = f8(inputs["s_w2"]), f8(inputs["s_b2"])
    s_wo, s_bo = f8(inputs["s_wo"]), f8(inputs["s_bo"])
    c_wq, c_bq = f8(inputs["c_wq"]), f8(inputs["c_bq"])
    c_wk, c_bk = f8(inputs["c_wk"]), f8(inputs["c_bk"])
    c_wo, c_bo = f8(inputs["c_wo"]), f8(inputs["c_bo"])
    f_w, f_b = f8(inputs["f_w"]), f8(inputs["f_b"])

    wt1, wt2 = wt[:CH], wt[CH:]
    wb1, wb2 = wb[:CH], wb[CH:]
    btb = bt + bb
    btb1, btb2 = btb[:CH], btb[CH:]

    A_q, B_q = s_w1 @ wt1, s_w1 @ wb1
    A_k, B_k = s_w2 @ wt1, s_w2 @ wb1
    C_q, D_q = c_wq @ wt2, c_wq @ wb2
    C_k, D_k = c_wk @ wt2, c_wk @ wb2

    wsp_full = np.concatenate(
        [
            np.concatenate([A_q.T, A_k.T], axis=1),
            np.concatenate([B_q.T, B_k.T], axis=1),
        ],
        axis=0,
    )  # [1024, 128]
    wsp = wsp_full.reshape(8, 128, 128).transpose(1, 0, 2)

    bias_q = s_w1 @ btb1 + s_b1
    bias_k = s_w2 @ btb1 + s_b2
    b_qk = np.concatenate([bias_q, bias_k])[:, None]

    wcsc_full = np.concatenate(
        [
            np.concatenate([C_q.T, C_k.T, A_k.T], axis=1),
            np.concatenate([D_q.T, D_k.T, B_k.T], axis=1),
        ],
        axis=0,
    )  # [1024, 192]
    wb_width = 192 if INPUT_BF16 else 256
    wcsc = np.zeros((8, 128, wb_width), np.float64)
    wcsc[:, :, :192] = wcsc_full.reshape(8, 128, 192)
    wcsc = wcsc.transpose(1, 0, 2)

    bias_qc = c_wq @ btb2 + c_bq
    bias_kc = c_wk @ btb2 + c_bk
    bcsc_vec = np.concatenate([bias_qc, bias_kc, bias_k])  # [192]
    b_csc = np.broadcast_to(bcsc_vec, (128, 192)).copy()
    b_kc = bias_kc[:, None]

    fs = f_w[:, :CH] @ s_wo  # [512, 64]
    fc = f_w[:, CH:] @ c_wo
    wfa = np.concatenate([fs, fc], axis=1).T.reshape(128, 4, 128)
    wfb = np.concatenate([fc, fs], axis=1).T.reshape(128, 4, 128)
    bias_f = f_w[:, :CH] @ s_bo + f_w[:, CH:] @ c_bo + f_b  # [512]
    b_f = bias_f.reshape(4, 128).T

    import ml_dtypes

    cast = lambda a: np.ascontiguousarray(a, dtype=np.float32)
    wdt = ml_dtypes.bfloat16 if INPUT_BF16 else np.float32
    wcast = lambda a: np.ascontiguousarray(a.astype(np.float32), dtype=wdt)
    return {
        "wsp": wcast(wsp),
        "wcsc": wcast(wcsc),
        "wfa": cast(wfa),
        "wfb": cast(wfb),
        "b_qk": cast(b_qk),
        "b_csc": cast(b_csc),
        "b_kc": cast(b_kc),
        "b_f": cast(b_f),
    }


def kernel(**inputs):
    if "nc" not in _CACHE:
        _CACHE["nc"] = build_program()
    nc = _CACHE["nc"]

    import ml_dtypes

    weights = make_weight_arrays(inputs)
    idt = ml_dtypes.bfloat16 if INPUT_BF16 else np.float32
    top_all = np.ascontiguousarray(
        np.asarray(inputs["top_feat"], dtype=np.float32)
        .reshape(N_CORES, C, HW)
        .astype(idt)
    )
    bot_all = np.ascontiguousarray(
        np.asarray(inputs["bottom_feat"], dtype=np.float32)
        .reshape(N_CORES, C, HW)
        .astype(idt)
    )
    in_maps = [
        {"top": top_all[b], "bot": bot_all[b], **weights} for b in range(N_CORES)
    ]
    res = bass_utils.run_bass_kernel_spmd(nc, in_maps, core_ids=list(range(N_CORES)))
    out = np.stack([res.results[b]["out"] for b in range(N_CORES)])
    return out.reshape(N_CORES, C, 64, 64)



# revision 2
# speedup vs baseline: 16520.8802x; 16520.8802x over previous
"""CKAM (DANet-style dual attention) Bass kernel for 8 trn2 NeuronCores.

Data-parallel over batch: each core processes one [512, 64, 64] image.

Per-core dataflow (N = H*W = 4096, C = 512, CH = 256, R = 64):
  Phase A: packed conv  [q|k](128, N)  = Wsp^T  @ [top;bottom]   (spatial q, k)
           chunk-outer accumulation, overlaps the input DMA stream.
  Phase B: transposed conv (N, 192) = [top;bottom]^T @ Wcsc      (qc^T, kc^T, ks^T)
  Phase C: conv          kc(64, N)  = Wkc^T @ [top;bottom]       (channel k)
  Channel attn:  scores = qc @ kc^T  (64x64), softmax, out_c = attn @ kc
  Spatial attn:  chunk pairs (even on PE rows 0:64, odd on rows 64:128 for
                 row-group concurrency): S = q^T k -> exp (ACT, accum d) ->
                 out_sp += (ks^T / d) contracted with E (col-group pairs)
  Final: out = [fs|fc] @ [out_sp; out_c] + bias   (single K=128 conv)

All 1x1 convs are folded through the (never materialized) x = top+bottom:
composite weights are computed on the host in float64.
"""

import numpy as np

import concourse.bass as bass
import concourse.bacc as bacc
import concourse.mybir as mybir
import concourse.tile as tile
from concourse import bass_utils
from concourse.bass import ts
from concourse.masks import make_identity

N_CORES = 8
C, HW = 512, 4096
CH, R = 256, 64
F32 = mybir.dt.float32
BF16 = mybir.dt.bfloat16
F32R = mybir.dt.float32r
EXP = mybir.ActivationFunctionType.Exp
AX = mybir.AxisListType.X

_CACHE: dict = {}

# Load top/bottom as bf16 (halves input DMA; rel err ~5e-3 vs ~2e-3)
INPUT_BF16 = True

ALL_PHASES = ("pa", "pb", "pc", "chan", "spatial", "final")


def build_program(phases=ALL_PHASES, repeat=1, input_bf16=None):
    if input_bf16 is None:
        input_bf16 = INPUT_BF16
    WDT = BF16 if input_bf16 else F32R
    WB = 192 if input_bf16 else 256
    IDT = BF16 if input_bf16 else F32R
    nc = bacc.Bacc("TRN2", target_bir_lowering=False, debug=False)

    top = nc.dram_tensor("top", (C, HW), IDT, kind="ExternalInput").ap()
    bot = nc.dram_tensor("bot", (C, HW), IDT, kind="ExternalInput").ap()
    wsp = nc.dram_tensor("wsp", (128, 8, 128), WDT, kind="ExternalInput").ap()
    wcsc = nc.dram_tensor("wcsc", (128, 8, WB), WDT, kind="ExternalInput").ap()
    wfa = nc.dram_tensor("wfa", (128, 4, 128), F32R, kind="ExternalInput").ap()
    wfb = nc.dram_tensor("wfb", (128, 4, 128), F32R, kind="ExternalInput").ap()
    b_qk = nc.dram_tensor("b_qk", (128, 1), F32, kind="ExternalInput").ap()
    b_csc = nc.dram_tensor("b_csc", (128, 192), F32, kind="ExternalInput").ap()
    b_kc = nc.dram_tensor("b_kc", (64, 1), F32, kind="ExternalInput").ap()
    b_f = nc.dram_tensor("b_f", (128, 4), F32, kind="ExternalInput").ap()
    out_d = nc.dram_tensor("out", (C, HW), F32, kind="ExternalOutput").ap()

    with tile.TileContext(nc) as tc:
      for _rep in range(repeat):
        with (
            tc.tile_pool(name="consts", bufs=1) as consts,
            tc.tile_pool(name="persist", bufs=1) as persist,
        ):
            wsp_sb = consts.tile([128, 8, 128], WDT)
            nc.sync.dma_start(out=wsp_sb, in_=wsp)
            wcsc_sb = consts.tile([128, 8, WB], WDT)
            nc.sync.dma_start(out=wcsc_sb, in_=wcsc)
            wfa_sb = consts.tile([128, 4, 128], F32R)
            nc.sync.dma_start(out=wfa_sb, in_=wfa)
            wfb_sb = consts.tile([128, 4, 128], F32R)
            nc.sync.dma_start(out=wfb_sb, in_=wfb)
            bqk_sb = consts.tile([128, 1], F32)
            nc.sync.dma_start(out=bqk_sb, in_=b_qk)
            bcsc_sb = consts.tile([128, 192], F32)
            nc.sync.dma_start(out=bcsc_sb, in_=b_csc)
            bkc_sb = consts.tile([64, 1], F32)
            nc.sync.dma_start(out=bkc_sb, in_=b_kc)
            bf_sb = consts.tile([128, 4], F32)
            nc.sync.dma_start(out=bf_sb, in_=b_f)
            ident = consts.tile([64, 64], F32)
            make_identity(nc, ident)

            # conv-phase outputs that live through the attention phase
            qk_sb = persist.tile([128, HW], BF16)  # q rows 0:64, k rows 64:128
            qk_swap = persist.tile([128, HW], BF16)  # [k | q] partition-swapped
            qckcT = persist.tile([128, 32, 128], F32)  # qc^T | kc^T  (n-major)
            ksT = persist.tile([128, 32, 64], BF16)  # spatial k^T
            kc_sb = persist.tile([64, HW], BF16)  # channel k
            stacked = persist.tile([128, HW], F32R)  # [out_sp|out_c] (swapped odd mb)

            # ---------------- conv phases (inputs resident) ----------------
            with tc.tile_pool(name="inputs", bufs=1) as inputs:
                top_r = top.rearrange("(a p) m -> a p m", p=128)
                bot_r = bot.rearrange("(a p) m -> a p m", p=128)
                srcs = [top_r[a] for a in range(4)] + [bot_r[a] for a in range(4)]
                if input_bf16:
                    chunks = []
                    for ci in range(8):
                        ch = inputs.tile([128, HW], BF16, tag=f"ch{ci}",
                                         name=f"ch{ci}")
                        nc.sync.dma_start(out=ch, in_=srcs[ci])
                        chunks.append(ch)
                else:
                    top_sb = inputs.tile([128, 4, HW], F32R)
                    bot_sb = inputs.tile([128, 4, HW], F32R)
                    for a in range(4):
                        nc.sync.dma_start(out=top_sb[:, a, :], in_=top_r[a])
                        nc.sync.dma_start(out=bot_sb[:, a, :], in_=bot_r[a])
                    chunks = [top_sb[:, a, :] for a in range(4)] + [
                        bot_sb[:, a, :] for a in range(4)
                    ]

                # Phase A (chunk-outer: starts as soon as chunk 0 lands)
                if "pa" in phases:
                    with tc.tile_pool(name="psA", bufs=1, space="PSUM") as psA:
                        psa_t = [
                            psA.tile([128, 512], F32, tag=f"a{mb}", name=f"psa{mb}")
                            for mb in range(8)
                        ]
                        for ci in range(8):
                            for mb in range(8):
                                nc.tensor.matmul(
                                    psa_t[mb],
                                    wsp_sb[:, ci, :],
                                    chunks[ci][:, ts(mb, 512)],
                                    start=(ci == 0),
                                    stop=(ci == 7),
                                )
                        for mb in range(8):
                            nc.vector.tensor_scalar_add(
                                qk_sb[:, ts(mb, 512)], psa_t[mb], bqk_sb
                            )
                    # [k|q] partition-swapped duplicate (SBUF->SBUF DMA)
                    nc.sync.dma_start(out=qk_swap[0:64, :], in_=qk_sb[64:128, :])
                    nc.sync.dma_start(out=qk_swap[64:128, :], in_=qk_sb[0:64, :])

                # Phase B: transposed convs -> qc^T | kc^T | ks^T
                if "pb" in phases:
                    with tc.tile_pool(name="psB", bufs=4, space="PSUM") as psB:
                        for nb in range(32):
                            ps = psB.tile([128, WB], F32, tag="b")
                            for ci in range(8):
                                nc.tensor.matmul(
                                    ps,
                                    chunks[ci][:, ts(nb, 128)],
                                    wcsc_sb[:, ci, :],
                                    start=(ci == 0),
                                    stop=(ci == 7),
                                )
                            nc.vector.tensor_add(
                                qckcT[:, nb, :], ps[:, 0:128], bcsc_sb[:, 0:128]
                            )
                            nc.vector.tensor_add(
                                ksT[:, nb, :], ps[:, 128:192], bcsc_sb[:, 128:192]
                            )

                # Phase C: channel kc conv -> [64, HW]
                if "pc" in phases:
                    with tc.tile_pool(name="psC", bufs=4, space="PSUM") as psC:
                        for mb in range(8):
                            ps = psC.tile([64, 512], F32, tag="c")
                            for ci in range(8):
                                nc.tensor.matmul(
                                    ps,
                                    wcsc_sb[:, ci, 64:128],
                                    chunks[ci][:, ts(mb, 512)],
                                    start=(ci == 0),
                                    stop=(ci == 7),
                                )
                            nc.vector.tensor_scalar_add(
                                kc_sb[:, ts(mb, 512)], ps, bkc_sb
                            )

            # ---------------- channel attention ----------------
            if "chan" not in phases:
                nc.vector.memset(stacked.bitcast(F32), 0.0)
            if "chan" in phases:
                with (
                    tc.tile_pool(name="chan", bufs=1) as chs,
                    tc.tile_pool(name="chp", bufs=1, space="PSUM") as chp,
                    tc.tile_pool(name="chop", bufs=2, space="PSUM") as chop,
                ):
                    sc_ps = chp.tile([64, 64], F32, tag="sc")
                    for nb in range(32):
                        nc.tensor.matmul(
                            sc_ps,
                            qckcT[:, nb, 0:64],
                            qckcT[:, nb, 64:128],
                            start=(nb == 0),
                            stop=(nb == 31),
                        )
                    sc = chs.tile([64, 64], F32)
                    nc.vector.tensor_copy(sc, sc_ps)
                    mx = chs.tile([64, 1], F32)
                    nc.vector.reduce_max(mx, sc, axis=AX)
                    negmx = chs.tile([64, 1], F32)
                    nc.vector.tensor_scalar_mul(negmx, mx, -1.0)
                    ec = chs.tile([64, 64], F32)
                    dc = chs.tile([64, 1], F32)
                    nc.scalar.activation(
                        ec, sc, EXP, bias=negmx, scale=1.0, accum_out=dc
                    )
                    rdc = chs.tile([64, 1], F32)
                    nc.vector.reciprocal(rdc, dc)
                    ac = chs.tile([64, 64], F32)
                    nc.vector.tensor_scalar_mul(ac, ec, rdc)
                    acT_ps = chp.tile([64, 64], F32, tag="acT")
                    nc.tensor.transpose(acT_ps, ac, ident)
                    acT = chs.tile([64, 64], BF16)
                    nc.vector.tensor_copy(acT, acT_ps)
                    # out_c: even mb -> partitions 64:128, odd mb -> 0:64
                    for j in range(4):
                        ps = chop.tile([128, 512], F32, tag="oc")
                        nc.tensor.matmul(
                            ps[64:128, :], acT, kc_sb[:, ts(2 * j, 512)],
                            start=True, stop=True, skip_group_check=True,
                        )
                        nc.tensor.matmul(
                            ps[0:64, :], acT, kc_sb[:, ts(2 * j + 1, 512)],
                            start=True, stop=True, skip_group_check=True,
                        )
                        nc.vector.tensor_copy(
                            stacked[64:128, ts(2 * j, 512)], ps[64:128, :]
                        )
                        nc.vector.tensor_copy(
                            stacked[0:64, ts(2 * j + 1, 512)], ps[0:64, :]
                        )

            # ---------------- spatial attention ----------------
            if "spatial" in phases:
                with (
                    tc.tile_pool(name="spE", bufs=2) as spp,
                    tc.tile_pool(name="spS", bufs=2) as sps,
                    tc.tile_pool(name="psSe", bufs=1, space="PSUM") as psSe,
                    tc.tile_pool(name="psSo", bufs=1, space="PSUM") as psSo,
                    tc.tile_pool(name="psO", bufs=1, space="PSUM") as psO,
                ):
                    out_ps = [
                        psO.tile([128, 512], F32, tag=f"o{j}", name=f"out_ps{j}")
                        for j in range(4)
                    ]
                    # chunk pairs: even chunk on PE rows 0:64, odd on 64:128
                    for t in range(16):
                        i_e, i_o = 2 * t, 2 * t + 1
                        E_e = spp.tile([128, HW], BF16, tag="Ee", name="E_e")
                        E_o = spp.tile([128, HW], BF16, tag="Eo", name="E_o")
                        dp_e = sps.tile([128, 4], F32, tag="dpe", name="dp_e")
                        dp_o = sps.tile([128, 4], F32, tag="dpo", name="dp_o")
                        for q in range(4):
                            s_e = psSe.tile([128, 1024], F32, tag="se", name="s_e")
                            s_o = psSo.tile([128, 1024], F32, tag="so", name="s_o")
                            for jm in range(2):
                                mb = 2 * q + jm
                                nc.tensor.matmul(
                                    s_e[:, ts(jm, 512)],
                                    qk_sb[0:64, ts(i_e, 128)],
                                    qk_swap[0:64, ts(mb, 512)],
                                    start=True,
                                    stop=True,
                                )
                                nc.tensor.matmul(
                                    s_o[:, ts(jm, 512)],
                                    qk_swap[64:128, ts(i_o, 128)],
                                    qk_sb[64:128, ts(mb, 512)],
                                    start=True,
                                    stop=True,
                                )
                            nc.scalar.activation(
                                E_e[:, ts(q, 1024)], s_e, EXP,
                                accum_out=dp_e[:, q : q + 1],
                            )
                            nc.scalar.activation(
                                E_o[:, ts(q, 1024)], s_o, EXP,
                                accum_out=dp_o[:, q : q + 1],
                            )
                        for par, i_c, dp, E in (
                            ("e", i_e, dp_e, E_e),
                            ("o", i_o, dp_o, E_o),
                        ):
                            d = sps.tile([128, 1], F32, tag=f"d{par}", name="d")
                            nc.vector.reduce_sum(d, dp, axis=AX)
                            rd = sps.tile([128, 1], F32, tag=f"rd{par}", name="rd")
                            nc.vector.reciprocal(rd, d)
                            kst = sps.tile(
                                [128, 64], BF16, tag=f"kst{par}", name="kst"
                            )
                            nc.vector.tensor_scalar_mul(kst, ksT[:, i_c, :], rd)
                            for j in range(4):
                                nc.tensor.matmul(
                                    out_ps[j][0:64, :], kst, E[:, ts(2 * j, 512)],
                                    start=(i_c == 0), stop=(i_c == 31),
                                    skip_group_check=True,
                                )
                                nc.tensor.matmul(
                                    out_ps[j][64:128, :],
                                    kst,
                                    E[:, ts(2 * j + 1, 512)],
                                    start=(i_c == 0), stop=(i_c == 31),
                                    skip_group_check=True,
                                )
                    for j in range(4):
                        nc.vector.tensor_copy(
                            stacked[0:64, ts(2 * j, 512)], out_ps[j][0:64, :]
                        )
                        nc.vector.tensor_copy(
                            stacked[64:128, ts(2 * j + 1, 512)],
                            out_ps[j][64:128, :],
                        )

            # ---------------- final fused conv ----------------
            if "final" in phases:
                with (
                    tc.tile_pool(name="fin", bufs=4) as fins,
                    tc.tile_pool(name="psF", bufs=4, space="PSUM") as psF,
                ):
                    out_r = out_d.rearrange("(k p) m -> k p m", p=128)
                    for mb in range(8):
                        wf = wfa_sb if mb % 2 == 0 else wfb_sb
                        for cok in range(4):
                            ps = psF.tile([128, 512], F32, tag="f")
                            nc.tensor.matmul(
                                ps,
                                wf[:, cok, :],
                                stacked[:, ts(mb, 512)],
                                start=True, stop=True,
                            )
                            ft = fins.tile([128, 512], F32, tag="ft")
                            if (mb * 4 + cok) % 2 == 0:
                                nc.vector.tensor_scalar_add(
                                    ft, ps, bf_sb[:, cok : cok + 1]
                                )
                            else:
                                nc.scalar.add(ft, ps, bf_sb[:, cok : cok + 1])
                            nc.sync.dma_start(
                                out=out_r[cok, :, ts(mb, 512)], in_=ft
                            )

    nc.compile()
    return nc


def make_weight_arrays(inputs):
    """Host-side composite weights (float64 accumulate, float32 out)."""
    f8 = lambda a: np.asarray(a, dtype=np.float64)
    wt, wb = f8(inputs["wt"]), f8(inputs["wb"])
    bt, bb = f8(inputs["bt"]), f8(inputs["bb"])
    s_w1, s_b1 = f8(inputs["s_w1"]), f8(inputs["s_b1"])
    s_w2, s_b2 = f8(inputs["s_w2"]), f8(inputs["s_b2"])
    s_wo, s_bo = f8(inputs["s_wo"]), f8(inputs["s_bo"])
    c_wq, c_bq = f8(inputs["c_wq"]), f8(inputs["c_bq"])
    c_wk, c_bk = f8(inputs["c_wk"]), f8(inputs["c_bk"])
    c_wo, c_bo = f8(inputs["c_wo"]), f8(inputs["c_bo"])
    f_w, f_b = f8(inputs["f_w"]), f8(inputs["f_b"])

    wt1, wt2 = wt[:CH], wt[CH:]
    wb1, wb2 = wb[:CH], wb[CH:]
    btb = bt + bb
    btb1, btb2 = btb[:CH], btb[CH:]

    A_q, B_q = s_w1 @ wt1, s_w1 @ wb1
    A_k, B_k = s_w2 @ wt1, s_w2 @ wb1
    C_q, D_q = c_wq @ wt2, c_wq @ wb2
    C_k, D_k = c_wk @ wt2, c_wk @ wb2

    wsp_full = np.concatenate(
        [
            np.concatenate([A_q.T, A_k.T], axis=1),
            np.concatenate([B_q.T, B_k.T], axis=1),
        ],
        axis=0,
    )  # [1024, 128]
    wsp = wsp_full.reshape(8, 128, 128).transpose(1, 0, 2)

    bias_q = s_w1 @ btb1 + s_b1
    bias_k = s_w2 @ btb1 + s_b2
    b_qk = np.concatenate([bias_q, bias_k])[:, None]

    wcsc_full = np.concatenate(
        [
            np.concatenate([C_q.T, C_k.T, A_k.T], axis=1),
            np.concatenate([D_q.T, D_k.T, B_k.T], axis=1),
        ],
        axis=0,
    )  # [1024, 192]
    wb_width = 192 if INPUT_BF16 else 256
    wcsc = np.zeros((8, 128, wb_width), np.float64)
    wcsc[:, :, :192] = wcsc_full.reshape(8, 128, 192)
    wcsc = wcsc.transpose(1, 0, 2)

    bias_qc = c_wq @ btb2 + c_bq
    bias_kc = c_wk @ btb2 + c_bk
    bcsc_vec = np.concatenate([bias_qc, bias_kc, bias_k])  # [192]
    b_csc = np.broadcast_to(bcsc_vec, (128, 192)).copy()
    b_kc = bias_kc[:, None]

    fs = f_w[:, :CH] @ s_wo  # [512, 64]
    fc = f_w[:, CH:] @ c_wo
    wfa = np.concatenate([fs, fc], axis=1).T.reshape(128, 4, 128)
    wfb = np.concatenate([fc, fs], axis=1).T.reshape(128, 4, 128)
    bias_f = f_w[:, :CH] @ s_bo + f_w[:, CH:] @ c_bo + f_b  # [512]
    b_f = bias_f.reshape(4, 128).T

    import ml_dtypes

    cast = lambda a: np.ascontiguousarray(a, dtype=np.float32)
    wdt = ml_dtypes.bfloat16 if INPUT_BF16 else np.float32
    wcast = lambda a: np.ascontiguousarray(a.astype(np.float32), dtype=wdt)
    return {
        "wsp": wcast(wsp),
        "wcsc": wcast(wcsc),
        "wfa": cast(wfa),
        "wfb": cast(wfb),
        "b_qk": cast(b_qk),
        "b_csc": cast(b_csc),
        "b_kc": cast(b_kc),
        "b_f": cast(b_f),
    }


def kernel(**inputs):
    if "nc" not in _CACHE:
        _CACHE["nc"] = build_program()
    nc = _CACHE["nc"]

    import ml_dtypes

    weights = make_weight_arrays(inputs)
    idt = ml_dtypes.bfloat16 if INPUT_BF16 else np.float32
    top_all = np.ascontiguousarray(
        np.asarray(inputs["top_feat"], dtype=np.float32)
        .reshape(N_CORES, C, HW)
        .astype(idt)
    )
    bot_all = np.ascontiguousarray(
        np.asarray(inputs["bottom_feat"], dtype=np.float32)
        .reshape(N_CORES, C, HW)
        .astype(idt)
    )
    in_maps = [
        {"top": top_all[b], "bot": bot_all[b], **weights} for b in range(N_CORES)
    ]
    res = bass_utils.run_bass_kernel_spmd(nc, in_maps, core_ids=list(range(N_CORES)))
    _CACHE["last_res"] = res
    out = np.stack([res.results[b]["out"] for b in range(N_CORES)])
    return out.reshape(N_CORES, C, 64, 64)

